# revision 1
# baseline (speedup 1.0000x reference)
"""Trainium2 Bass kernel for KnowledgeAugmentedFusion.

  v = visual @ Wv.T + bv                      [B, D]
  t = text @ Wt.T + bt                        [B, D]
  k = knowledge @ Wk.T + bk                   [B, D]
  s = einsum('bj,ijl,bl->bi', t, W3, k)       [B, D]   (W3: [D, D, D])
  out = LayerNorm((v * s) @ Wo.T + bo)        [B, D]

Sharding: W3 along output-channel axis i across 8 cores (64 rows each).
Per core, per i:  psum[b, l] = sum_j t[b, j] * W3[i, j, l]  (4 bf16 matmuls,
natural W3 layout), then s[b, i] = sum_l psum[b, l] * k[b, l] via one fused
tensor_tensor_reduce on DVE.  fused=v*s slices are AllGathered, and every
core runs the (tiny) output-layer + LayerNorm epilogue redundantly.

W3 is cast to bf16 on the host (memory-bound kernel -> halves HBM traffic;
matmul accumulation stays fp32 in PSUM).
"""

import sys

if "/opt/trn_rl_repo" not in sys.path:
    sys.path.insert(0, "/opt/trn_rl_repo")

import numpy as np
import ml_dtypes

B = 16
VD, TD, KD, D = 2048, 768, 1024, 512
NCORES = 8
DSH = D // NCORES  # 64 output channels per core
LN_EPS = 1e-5

BF16 = ml_dtypes.bfloat16

_CACHE = {}
LAST = {}


def _build_module(w3_bufs=8):
    import os
    n_i = int(os.environ.get("K_NI", str(DSH)))
    use_cc = os.environ.get("K_CC", "1") == "1"
    use_epi = os.environ.get("K_EPI", "1") == "1"
    from concourse import bacc, tile, mybir

    fp32 = mybir.dt.float32
    bf16 = mybir.dt.bfloat16
    AX = mybir.AxisListType
    OP = mybir.AluOpType
    ACT = mybir.ActivationFunctionType

    nc = bacc.Bacc("TRN2", target_bir_lowering=False, debug=False,
                   num_devices=NCORES)

    # ---- DRAM I/O ----------------------------------------------------
    w3s = nc.dram_tensor("w3s", [DSH, D, D], bf16, kind="ExternalInput")
    wtT = nc.dram_tensor("wtT", [TD, D], bf16, kind="ExternalInput")
    wkT = nc.dram_tensor("wkT", [KD, D], bf16, kind="ExternalInput")
    wvTs = nc.dram_tensor("wvTs", [VD, DSH], fp32, kind="ExternalInput")
    woT = nc.dram_tensor("woT", [D, D], fp32, kind="ExternalInput")
    textT = nc.dram_tensor("textT", [TD, B], bf16, kind="ExternalInput")
    knowT = nc.dram_tensor("knowT", [KD, B], bf16, kind="ExternalInput")
    visT = nc.dram_tensor("visT", [VD, B], fp32, kind="ExternalInput")
    btT = nc.dram_tensor("btT", [D, 1], fp32, kind="ExternalInput")
    bv_rep = nc.dram_tensor("bv_rep", [B, DSH], fp32, kind="ExternalInput")
    bk_rep = nc.dram_tensor("bk_rep", [B, D], fp32, kind="ExternalInput")
    bo_rep = nc.dram_tensor("bo_rep", [B, D], fp32, kind="ExternalInput")
    g_rep = nc.dram_tensor("g_rep", [B, D], fp32, kind="ExternalInput")
    be_rep = nc.dram_tensor("be_rep", [B, D], fp32, kind="ExternalInput")
    out = nc.dram_tensor("out", [B, D], fp32, kind="ExternalOutput")
    dbg = nc.dram_tensor("dbg", [B, DSH], fp32, kind="ExternalOutput")

    with tile.TileContext(nc) as tc:
        with tc.tile_pool(name="const", bufs=1) as constp, \
             tc.tile_pool(name="w3p", bufs=w3_bufs) as w3p, \
             tc.tile_pool(name="scr", bufs=2) as scrp, \
             tc.tile_pool(name="pp", bufs=2, space="PSUM") as pp, \
             tc.tile_pool(name="pi", bufs=4, space="PSUM") as pip, \
             tc.tile_pool(name="dram", bufs=1, space="DRAM") as dramp:

            # ---- weights/constants into SBUF -------------------------
            wtT_sb = constp.tile([128, 6 * D], bf16)
            nc.sync.dma_start(out=wtT_sb[:].rearrange("p (c d) -> p c d", c=6),
                              in_=wtT.ap().rearrange("(c p) d -> p c d", p=128))
            wkT_sb = constp.tile([128, 8 * D], bf16)
            nc.sync.dma_start(out=wkT_sb[:].rearrange("p (c d) -> p c d", c=8),
                              in_=wkT.ap().rearrange("(c p) d -> p c d", p=128))
            wvTs_sb = constp.tile([128, 16 * DSH], fp32)
            nc.sync.dma_start(out=wvTs_sb[:].rearrange("p (c d) -> p c d", c=16),
                              in_=wvTs.ap().rearrange("(c p) d -> p c d", p=128))
            textT_sb = constp.tile([128, 6 * B], bf16)
            nc.sync.dma_start(out=textT_sb[:].rearrange("p (c b) -> p c b", c=6),
                              in_=textT.ap().rearrange("(c p) b -> p c b", p=128))
            knowT_sb = constp.tile([128, 8 * B], bf16)
            nc.sync.dma_start(out=knowT_sb[:].rearrange("p (c b) -> p c b", c=8),
                              in_=knowT.ap().rearrange("(c p) b -> p c b", p=128))
            visT_sb = constp.tile([128, 16 * B], fp32)
            nc.sync.dma_start(out=visT_sb[:].rearrange("p (c b) -> p c b", c=16),
                              in_=visT.ap().rearrange("(c p) b -> p c b", p=128))
            btT_sb = constp.tile([128, 4], fp32)
            nc.sync.dma_start(out=btT_sb[:].rearrange("p (m o) -> p m o", m=4),
                              in_=btT.ap().rearrange("(m p) o -> p m o", p=128))
            bk_sb = constp.tile([B, D], fp32)
            nc.sync.dma_start(out=bk_sb[:], in_=bk_rep.ap())
            bv_sb = constp.tile([B, DSH], fp32)
            nc.sync.dma_start(out=bv_sb[:], in_=bv_rep.ap())
            bo_sb = constp.tile([B, D], fp32)
            nc.sync.dma_start(out=bo_sb[:], in_=bo_rep.ap())
            g_sb = constp.tile([B, D], fp32)
            nc.sync.dma_start(out=g_sb[:], in_=g_rep.ap())
            be_sb = constp.tile([B, D], fp32)
            nc.sync.dma_start(out=be_sb[:], in_=be_rep.ap())
            woT_sb = constp.tile([128, 4 * D], fp32)
            nc.sync.dma_start(out=woT_sb[:].rearrange("p (c d) -> p c d", c=4),
                              in_=woT.ap().rearrange("(c p) d -> p c d", p=128))

            # ---- t = text @ Wt.T + bt, as tT [512j, 16b] bf16 --------
            tT_sb = constp.tile([128, 4 * B], bf16)
            for mt in range(4):
                ps_t = pp.tile([128, B], fp32, tag="pp")
                for ct in range(6):
                    nc.tensor.matmul(
                        out=ps_t[:],
                        lhsT=wtT_sb[:, D * ct + 128 * mt: D * ct + 128 * mt + 128],
                        rhs=textT_sb[:, B * ct: B * ct + B],
                        start=(ct == 0), stop=(ct == 5))
                nc.vector.tensor_scalar(
                    out=tT_sb[:, B * mt: B * mt + B], in0=ps_t[:],
                    scalar1=btT_sb[:, mt: mt + 1], scalar2=None, op0=OP.add)

            # ---- k = knowledge @ Wk.T + bk, natural [16b, 512l] ------
            ps_k = pp.tile([B, D], fp32, tag="pp")
            for ct in range(8):
                nc.tensor.matmul(
                    out=ps_k[:],
                    lhsT=knowT_sb[:, B * ct: B * ct + B],
                    rhs=wkT_sb[:, D * ct: D * ct + D],
                    start=(ct == 0), stop=(ct == 7))
            k_sb = constp.tile([B, D], fp32)
            nc.vector.tensor_tensor(out=k_sb[:], in0=ps_k[:], in1=bk_sb[:],
                                    op=OP.add)

            # ---- v slice = visual @ WvT[:, shard] + bv, [16b, 64i] ---
            ps_v = pp.tile([B, DSH], fp32, tag="pp")
            for ct in range(16):
                nc.tensor.matmul(
                    out=ps_v[:],
                    lhsT=visT_sb[:, B * ct: B * ct + B],
                    rhs=wvTs_sb[:, DSH * ct: DSH * ct + DSH],
                    start=(ct == 0), stop=(ct == 15))
            v_sb = constp.tile([B, DSH], fp32)
            nc.vector.tensor_tensor(out=v_sb[:], in0=ps_v[:], in1=bv_sb[:],
                                    op=OP.add)

            # ---- main loop: s[:, i] for each local output channel ----
            S_cols = constp.tile([B, DSH], fp32)
            nc.vector.memset(S_cols[:], 0.0)
            body = os.environ.get("K_BODY", "ttr")
            for i in range(n_i):
                w3t = w3p.tile([128, 4 * D], bf16, tag="w3t")
                nc.sync.dma_start(
                    out=w3t[:].rearrange("p (jt l) -> p jt l", jt=4),
                    in_=w3s.ap()[i].rearrange("(jt p) l -> p jt l", p=128))
                if body == "dma":
                    continue
                ps = pip.tile([B, D], fp32, tag="ps")
                for jt in range(4):
                    nc.tensor.matmul(
                        out=ps[:],
                        lhsT=tT_sb[:, B * jt: B * jt + B],
                        rhs=w3t[:, D * jt: D * jt + D],
                        start=(jt == 0), stop=(jt == 3))
                if body == "mm":
                    junk = scrp.tile([B, D], fp32, tag="junk")
                    nc.vector.tensor_copy(junk[:], ps[:])
                    continue
                prod = scrp.tile([B, D], fp32, tag="prod")
                nc.vector.tensor_tensor(out=prod[:], in0=ps[:], in1=k_sb[:],
                                        op=OP.mult)
                junk = scrp.tile([B, D], fp32, tag="junk")
                nc.scalar.activation(out=junk[:], in_=prod[:],
                                     func=ACT.Copy,
                                     accum_out=S_cols[:, i: i + 1])

            # ---- fused = v * s  [16, 64], all-gather over cores ------
            fused_sb = constp.tile([B, DSH], fp32)
            nc.vector.tensor_tensor(out=fused_sb[:], in0=v_sb[:],
                                    in1=S_cols[:], op=OP.mult)
            nc.sync.dma_start(out=dbg.ap(), in_=fused_sb[:])
            if use_cc:
                cc_in = dramp.tile([B, DSH], fp32)
                nc.sync.dma_start(out=cc_in[:], in_=fused_sb[:])
                cc_out = dramp.tile([NCORES, B, DSH], fp32)
                nc.gpsimd.collective_compute(
                    "AllGather", OP.bypass,
                    replica_groups=[list(range(NCORES))],
                    ins=[cc_in.opt()], outs=[cc_out.opt()])

            # fusedT [512i, 16b] as [128, (4it, 16b)]; i = 128*it + p
            fusedT_sb = constp.tile([128, 4 * B], fp32)
            if use_cc:
                for c in range(NCORES):
                    nc.sync.dma_start(
                        out=fusedT_sb[64 * (c % 2): 64 * (c % 2) + 64,
                                      B * (c // 2): B * (c // 2) + B],
                        in_=cc_out[:][c].transpose([1, 0]))
            else:
                nc.vector.memset(fusedT_sb[:], 0.0)

            if use_epi:
                # ---- epilogue: out = LN(fused @ Wo.T + bo) ---------------
                ps_o = pp.tile([B, D], fp32, tag="pp")
                for it in range(4):
                    nc.tensor.matmul(
                        out=ps_o[:],
                        lhsT=fusedT_sb[:, B * it: B * it + B],
                        rhs=woT_sb[:, D * it: D * it + D],
                        start=(it == 0), stop=(it == 3))
                x_sb = scrp.tile([B, D], fp32, tag="x")
                nc.vector.tensor_tensor(out=x_sb[:], in0=ps_o[:], in1=bo_sb[:],
                                        op=OP.add)
                sum_t = constp.tile([B, 1], fp32)
                nc.vector.tensor_reduce(out=sum_t[:], in_=x_sb[:], axis=AX.X,
                                        op=OP.add)
                mean_t = constp.tile([B, 1], fp32)
                nc.scalar.mul(mean_t[:], sum_t[:], 1.0 / D)
                xc_sb = scrp.tile([B, D], fp32, tag="xc")
                nc.vector.tensor_scalar(out=xc_sb[:], in0=x_sb[:],
                                        scalar1=mean_t[:], scalar2=None,
                                        op0=OP.subtract)
                sq_sb = scrp.tile([B, D], fp32, tag="sq")
                var_t = constp.tile([B, 1], fp32)
                zero_t = constp.tile([B, 1], fp32)
                nc.vector.memset(zero_t[:], 0.0)
                nc.scalar.activation(out=sq_sb[:], in_=xc_sb[:],
                                     func=ACT.Square, bias=zero_t[:],
                                     accum_out=var_t[:])
                eps_t = constp.tile([B, 1], fp32)
                nc.vector.memset(eps_t[:], LN_EPS)
                std_t = constp.tile([B, 1], fp32)
                nc.scalar.activation(out=std_t[:], in_=var_t[:], func=ACT.Sqrt,
                                     bias=eps_t[:], scale=1.0 / D)
                rstd_t = constp.tile([B, 1], fp32)
                nc.vector.reciprocal(out=rstd_t[:], in_=std_t[:])
                xn_sb = scrp.tile([B, D], fp32, tag="xn")
                nc.vector.tensor_scalar(out=xn_sb[:], in0=xc_sb[:],
                                        scalar1=rstd_t[:], scalar2=None,
                                        op0=OP.mult)
                y_sb = scrp.tile([B, D], fp32, tag="y")
                nc.vector.tensor_tensor(out=y_sb[:], in0=xn_sb[:], in1=g_sb[:],
                                        op=OP.mult)
                out_sb = scrp.tile([B, D], fp32, tag="o")
                nc.vector.tensor_tensor(out=out_sb[:], in0=y_sb[:], in1=be_sb[:],
                                        op=OP.add)
                nc.sync.dma_start(out=out.ap(), in_=out_sb[:])
            else:
                nc.sync.dma_start(out=out.ap(), in_=be_sb[:])

    nc.compile()
    return nc


def _prep_in_maps(inputs):
    f32 = np.float32

    def cvt(x, dt):
        return np.ascontiguousarray(np.asarray(x), dtype=dt)

    W3 = np.asarray(inputs["W3"], dtype=f32)
    WvT = np.ascontiguousarray(np.asarray(inputs["Wv"], dtype=f32).T)
    bv = np.asarray(inputs["bv"], dtype=f32)

    shared = {
        "wtT": cvt(np.asarray(inputs["Wt"], dtype=f32).T, BF16),
        "wkT": cvt(np.asarray(inputs["Wk"], dtype=f32).T, BF16),
        "woT": cvt(np.asarray(inputs["Wo"], dtype=f32).T, f32),
        "textT": cvt(np.asarray(inputs["text_features"], dtype=f32).T, BF16),
        "knowT": cvt(np.asarray(inputs["knowledge_features"], dtype=f32).T, BF16),
        "visT": cvt(np.asarray(inputs["visual_features"], dtype=f32).T, f32),
        "btT": cvt(np.asarray(inputs["bt"], dtype=f32).reshape(D, 1), f32),
        "bk_rep": np.tile(np.asarray(inputs["bk"], dtype=f32).reshape(1, D), (B, 1)),
        "bo_rep": np.tile(np.asarray(inputs["bo"], dtype=f32).reshape(1, D), (B, 1)),
        "g_rep": np.tile(np.asarray(inputs["gamma"], dtype=f32).reshape(1, D), (B, 1)),
        "be_rep": np.tile(np.asarray(inputs["beta"], dtype=f32).reshape(1, D), (B, 1)),
    }
    in_maps = []
    for m in range(NCORES):
        sl = slice(DSH * m, DSH * (m + 1))
        per = dict(shared)
        per["w3s"] = np.ascontiguousarray(W3[sl]).astype(BF16)
        per["wvTs"] = np.ascontiguousarray(WvT[:, sl])
        per["bv_rep"] = np.tile(bv[sl].reshape(1, DSH), (B, 1))
        in_maps.append(per)
    return in_maps


def kernel(**inputs):
    import os
    from concourse.bass_utils import run_bass_kernel_spmd

    if "nc" not in _CACHE:
        _CACHE["nc"] = _build_module()
    nc = _CACHE["nc"]

    in_maps = _prep_in_maps(inputs)
    trace = os.environ.get("KERNEL_TRACE", "0") == "1"
    res = run_bass_kernel_spmd(nc, in_maps, core_ids=list(range(NCORES)),
                               trace=trace)
    LAST["exec_time_ns"] = res.exec_time_ns
    LAST["results"] = res
    return np.asarray(res.results[0]["out"], dtype=np.float32)



# revision 11
# speedup vs baseline: 1.7681x; 1.7681x over previous
"""Trainium2 Bass kernel for KnowledgeAugmentedFusion.

  v = visual @ Wv.T + bv                      [B, D]
  t = text @ Wt.T + bt                        [B, D]
  k = knowledge @ Wk.T + bk                   [B, D]
  s = einsum('bj,ijl,bl->bi', t, W3, k)       [B, D]   (W3: [D, D, D])
  out = LayerNorm((v * s) @ Wo.T + bo)        [B, D]

Sharding: W3 along output-channel axis i across 8 cores (64 rows each);
fused = v*s slices are AllGathered, every core runs the small
output-layer + LayerNorm epilogue redundantly (per the sharding hint).

Dataflow (per core, per output channel i):
  stage 1:  psT[l, b] = sum_j W3[i, j, l] * t[b, j]
            -- W3 [128j x 128l] blocks are the STATIONARY matmul operand,
               tT [128j, 16b] the moving one; psT accumulates in one PSUM
               bank as 4 l-tile groups (onto a memset bank, start=False).
  stage 2:  prod[l, b] = psT[l, b] * kT[l, b]        (one DVE op, [128, 64])
  stage 3:  S4[(lt,b), i] = sum_p prod[p, (lt,b)]    (ones-matmul partition
            reduce into a persistent PSUM bank)
  end:      sT[i, b] = sum_lt S4[(lt,b), i]          (constant E matmul)
            fusedT = sT * vT, AllGather, epilogue matmul + LayerNorm.

W3 is streamed as a mix of bf16 and fp8 blocks (NF of the 16 [128x128]
(jt,lt) blocks per i are fp8): fp8 halves HBM traffic for those blocks;
the fp8 rounding error is tuned against the rel-err budget. A global
power-of-2 scale keeps fp8 values in the normal range; it is folded
into v on the host.
"""

import os
import sys

if "/opt/trn_rl_repo" not in sys.path:
    sys.path.insert(0, "/opt/trn_rl_repo")

import numpy as np
import ml_dtypes

B = 16
VD, TD, KD, D = 2048, 768, 1024, 512
NCORES = 8
DSH = D // NCORES  # 64 output channels per core
LN_EPS = 1e-5

BF16 = ml_dtypes.bfloat16

# --- W3 streaming config (env-overridable for experiments) --------------
NF = int(os.environ.get("K_NF", "16"))     # fp8 blocks out of 16 per i
NB = 16 - NF
F8 = os.environ.get("K_F8", "e3")          # e3 -> e3m4, e4 -> e4m3
G = int(os.environ.get("K_G", "2"))        # i-channels per W3 DMA
NG = DSH // G
W3SCALE = 256.0 if F8 == "e3" else 64.0
F8NP = ml_dtypes.float8_e3m4 if F8 == "e3" else ml_dtypes.float8_e4m3

# block order: first NB blocks bf16, last NF fp8 ((jt, lt) lexicographic)
ALL_BLOCKS = [(jt, lt) for jt in range(4) for lt in range(4)]
BF_BLOCKS = ALL_BLOCKS[:NB]
F8_BLOCKS = ALL_BLOCKS[NB:]
BF_POS = {b: n for n, b in enumerate(BF_BLOCKS)}
F8_POS = {b: n for n, b in enumerate(F8_BLOCKS)}

# aux blob column offsets (bf16, [128, AUXW])
OFF_WT = 0                      # [128, 6, 512]  Wt.T by (ct, j)
OFF_WK = OFF_WT + 6 * 512       # [128, 8, 512]  Wk.T by (ct, l)
OFF_WV = OFF_WK + 8 * 512       # [128, 16, 64]  Wv.T slice / W3SCALE
OFF_WO = OFF_WV + 16 * DSH      # [128, 4, 512]  Wo.T by (it, c)
OFF_WOS = OFF_WO + 4 * 512      # [128, 4]       col-sums of Wo.T by (it)
OFF_TX = OFF_WOS + 4            # [128, 6, 16]   text.T
OFF_KN = OFF_TX + 6 * B         # [128, 8, 16]   knowledge.T
OFF_VS = OFF_KN + 8 * B         # [128, 16, 16]  visual.T
AUXW = OFF_VS + 16 * B

_CACHE = {}
LAST = {}


def _build_module():
    n_i = int(os.environ.get("K_NI", str(DSH)))
    use_cc = os.environ.get("K_CC", "1") == "1"
    use_epi = os.environ.get("K_EPI", "1") == "1"
    w3_bufs = int(os.environ.get("K_W3BUFS", "4"))
    from concourse import bacc, tile, mybir

    fp32 = mybir.dt.float32
    bf16 = mybir.dt.bfloat16
    f8 = mybir.dt.float8e3 if F8 == "e3" else mybir.dt.float8e4
    OP = mybir.AluOpType
    ACT = mybir.ActivationFunctionType

    nc = bacc.Bacc("TRN2", target_bir_lowering=False, debug=False,
                   num_devices=NCORES)

    # ---- DRAM I/O ----------------------------------------------------
    XB = NB * 128
    XF = NF * 128
    if NB:
        w3bf = nc.dram_tensor("w3bf", [NG, G, 128, XB], bf16,
                              kind="ExternalInput")
    if NF:
        w3f8 = nc.dram_tensor("w3f8", [NG, G, 128, XF], f8,
                              kind="ExternalInput")
    aux = nc.dram_tensor("aux", [128, AUXW], bf16, kind="ExternalInput")
    smalls = nc.dram_tensor("smalls", [128, 28], fp32, kind="ExternalInput")
    reps = nc.dram_tensor("reps", [B, 3 * D], fp32, kind="ExternalInput")
    out = nc.dram_tensor("out", [B, D], fp32, kind="ExternalOutput")

    with tile.TileContext(nc) as tc:
        with tc.tile_pool(name="const", bufs=1) as constp, \
             tc.tile_pool(name="w3p", bufs=w3_bufs) as w3p, \
             tc.tile_pool(name="f8p", bufs=w3_bufs) as f8p, \
             tc.tile_pool(name="scr", bufs=2) as scrp, \
             tc.tile_pool(name="pp", bufs=2, space="PSUM") as pp, \
             tc.tile_pool(name="psq", bufs=2, space="PSUM") as psq, \
             tc.tile_pool(name="ps4", bufs=1, space="PSUM") as ps4, \
             tc.tile_pool(name="ppx", bufs=1, space="PSUM") as ppx, \
             tc.tile_pool(name="ppm", bufs=1, space="PSUM") as ppm, \
             tc.tile_pool(name="dram", bufs=1, space="DRAM") as dramp:

            # ---- aux / constants into SBUF ---------------------------
            aux_sb = constp.tile([128, AUXW], bf16)
            nc.sync.dma_start(out=aux_sb[:], in_=aux.ap())
            smalls_sb = constp.tile([128, 28], fp32)
            nc.sync.dma_start(out=smalls_sb[:], in_=smalls.ap())
            reps_sb = constp.tile([B, 3 * D], fp32)
            nc.sync.dma_start(out=reps_sb[:], in_=reps.ap())

            ones_sb = constp.tile([128, 1], fp32)
            nc.vector.memset(ones_sb[:], 1.0)

            # ---- prologue: tT (bf16), kT (fp32), vT (fp32) -----------
            tT_sb = constp.tile([128, 4 * B], bf16)
            for jt in range(4):
                pt = pp.tile([128, B], fp32, tag="pp")
                for ct in range(6):
                    nc.tensor.matmul(
                        out=pt[:],
                        lhsT=aux_sb[:, OFF_WT + ct * 512 + jt * 128:
                                    OFF_WT + ct * 512 + jt * 128 + 128],
                        rhs=aux_sb[:, OFF_TX + ct * B: OFF_TX + ct * B + B],
                        start=(ct == 0), stop=(ct == 5))
                nc.vector.tensor_scalar(
                    out=tT_sb[:, B * jt: B * jt + B], in0=pt[:],
                    scalar1=smalls_sb[:, jt: jt + 1], scalar2=None,
                    op0=OP.add)

            kT_sb = constp.tile([128, 4 * B], fp32)
            for lt in range(4):
                pk = pp.tile([128, B], fp32, tag="pp")
                for ct in range(8):
                    nc.tensor.matmul(
                        out=pk[:],
                        lhsT=aux_sb[:, OFF_WK + ct * 512 + lt * 128:
                                    OFF_WK + ct * 512 + lt * 128 + 128],
                        rhs=aux_sb[:, OFF_KN + ct * B: OFF_KN + ct * B + B],
                        start=(ct == 0), stop=(ct == 7))
                nc.vector.tensor_scalar(
                    out=kT_sb[:, B * lt: B * lt + B], in0=pk[:],
                    scalar1=smalls_sb[:, 4 + lt: 5 + lt], scalar2=None,
                    op0=OP.add)

            pv = pp.tile([DSH, B], fp32, tag="pp")
            for ct in range(16):
                nc.tensor.matmul(
                    out=pv[:],
                    lhsT=aux_sb[:, OFF_WV + ct * DSH: OFF_WV + ct * DSH + DSH],
                    rhs=aux_sb[:, OFF_VS + ct * B: OFF_VS + ct * B + B],
                    start=(ct == 0), stop=(ct == 15))
            vsb = constp.tile([DSH, B], fp32)
            nc.vector.tensor_scalar(
                out=vsb[:], in0=pv[:], scalar1=smalls_sb[0:DSH, 24:25],
                scalar2=None, op0=OP.add)

            # ---- persistent S4 accumulator [64 (lt,b), 64 i] ---------
            S4 = ps4.tile([64, 64], fp32)
            nc.vector.memset(S4[:], 0.0)

            # ---- main loop over output channels ----------------------
            for g in range(NG):
                if g * G >= n_i:
                    break
                if NB:
                    wbf = w3p.tile([128, G * XB], bf16, tag="wbf")
                    nc.sync.dma_start(
                        out=wbf[:].rearrange("p (g x) -> p g x", g=G),
                        in_=w3bf.ap()[g].rearrange("g p x -> p g x"))
                if NF:
                    wf8 = f8p.tile([128, G * XF], f8, tag="wf8")
                    nc.sync.dma_start(
                        out=wf8[:].rearrange("p (g x) -> p g x", g=G),
                        in_=w3f8.ap()[g].rearrange("g p x -> p g x"))
                for gi in range(G):
                    i = g * G + gi
                    psT = psq.tile([128, 64], fp32, tag="psT")
                    nc.vector.memset(psT[:], 0.0)
                    for lt in range(4):
                        for jt in range(4):
                            if (jt, lt) in BF_POS:
                                col = (gi * NB + BF_POS[(jt, lt)]) * 128
                                lhsT = wbf[:, col: col + 128]
                            else:
                                col = (gi * NF + F8_POS[(jt, lt)]) * 128
                                lhsT = wf8[:, col: col + 128]
                            nc.tensor.matmul(
                                out=psT[:, lt * B: lt * B + B],
                                lhsT=lhsT,
                                rhs=tT_sb[:, B * jt: B * jt + B],
                                start=False, stop=True,
                                skip_group_check=True)
                    prod = scrp.tile([128, 64], fp32, tag="prod")
                    nc.vector.tensor_tensor(out=prod[:], in0=psT[:],
                                            in1=kT_sb[:], op=OP.mult)
                    nc.tensor.matmul(
                        out=S4[:, i: i + 1], lhsT=prod[:], rhs=ones_sb[:],
                        start=False, stop=True, skip_group_check=True)

            # ---- s -> fused -> AllGather -----------------------------
            S4sb = constp.tile([64, 64], fp32)
            nc.vector.tensor_copy(S4sb[:], S4[:])
            sT = pp.tile([DSH, B], fp32, tag="pp")
            nc.tensor.matmul(out=sT[:], lhsT=S4sb[:],
                             rhs=smalls_sb[0:64, 8:24], start=True, stop=True)
            fusedT_bf = constp.tile([DSH, B], bf16)
            nc.vector.tensor_tensor(out=fusedT_bf[:], in0=sT[:], in1=vsb[:],
                                    op=OP.mult)

            cc_in = dramp.tile([DSH, B], bf16)
            nc.sync.dma_start(out=cc_in[:], in_=fusedT_bf[:])
            cc_out = dramp.tile([NCORES, DSH, B], bf16)
            if use_cc:
                nc.gpsimd.collective_compute(
                    "AllGather", OP.bypass,
                    replica_groups=[list(range(NCORES))],
                    ins=[cc_in.opt()], outs=[cc_out.opt()])

            # fusedT [512 i, 16 b] as [128, (4 it, 16 b)]; i = 128*it + p
            fusedT_sb = constp.tile([128, 4 * B], bf16)
            nc.sync.dma_start(
                out=fusedT_sb[:].rearrange("p (it b) -> p it b", it=4),
                in_=cc_out[:].rearrange("(it c2) r b -> (c2 r) it b", it=4))

            if use_epi:
                # ---- epilogue: out = LN(fused @ Wo.T + bo) -----------
                ps_x = ppx.tile([B, D], fp32, tag="ppx")
                ps_mu = ppm.tile([B, 1], fp32, tag="ppm")
                for it in range(4):
                    nc.tensor.matmul(
                        out=ps_x[:],
                        lhsT=fusedT_sb[:, B * it: B * it + B],
                        rhs=aux_sb[:, OFF_WO + it * 512: OFF_WO + it * 512 + 512],
                        start=(it == 0), stop=(it == 3))
                for it in range(4):
                    nc.tensor.matmul(
                        out=ps_mu[:],
                        lhsT=fusedT_sb[:, B * it: B * it + B],
                        rhs=aux_sb[:, OFF_WOS + it: OFF_WOS + it + 1],
                        start=(it == 0), stop=(it == 3))

                eps_t = constp.tile([B, 1], fp32)
                nc.vector.memset(eps_t[:], LN_EPS)

                # x = ps_x + bo ; mean from the colsum matmul
                x_sb = scrp.tile([B, D], fp32, tag="x")
                nc.vector.scalar_tensor_tensor(
                    out=x_sb[:], in0=ps_x[:], scalar=1.0,
                    in1=reps_sb[:, 0:D], op0=OP.mult, op1=OP.add)
                mean_t = constp.tile([B, 1], fp32)
                nc.vector.tensor_scalar(
                    out=mean_t[:], in0=ps_mu[:],
                    scalar1=smalls_sb[0:B, 25:26], scalar2=1.0 / D,
                    op0=OP.add, op1=OP.mult)
                # var = E[x^2] - mean^2
                sq_junk = scrp.tile([B, D], fp32, tag="sqj")
                sqs_t = constp.tile([B, 1], fp32)
                nc.scalar.activation(out=sq_junk[:], in_=x_sb[:],
                                     func=ACT.Square, accum_out=sqs_t[:])
                mu2_t = constp.tile([B, 1], fp32)
                nc.scalar.activation(out=mu2_t[:], in_=mean_t[:],
                                     func=ACT.Square)
                var_t = constp.tile([B, 1], fp32)
                nc.vector.scalar_tensor_tensor(
                    out=var_t[:], in0=sqs_t[:], scalar=1.0 / D,
                    in1=mu2_t[:], op0=OP.mult, op1=OP.subtract)
                std_t = constp.tile([B, 1], fp32)
                nc.scalar.activation(out=std_t[:], in_=var_t[:],
                                     func=ACT.Sqrt, bias=eps_t[:])
                rstd_t = constp.tile([B, 1], fp32)
                nc.vector.reciprocal(out=rstd_t[:], in_=std_t[:])
                # y = (x - mean) * gamma * rstd + beta
                xn_sb = scrp.tile([B, D], fp32, tag="xn")
                nc.vector.scalar_tensor_tensor(
                    out=xn_sb[:], in0=x_sb[:], scalar=mean_t[:],
                    in1=reps_sb[:, D:2 * D], op0=OP.subtract, op1=OP.mult)
                y_sb = scrp.tile([B, D], fp32, tag="y")
                nc.vector.scalar_tensor_tensor(
                    out=y_sb[:], in0=xn_sb[:], scalar=rstd_t[:],
                    in1=reps_sb[:, 2 * D:3 * D], op0=OP.mult, op1=OP.add)
                nc.sync.dma_start(out=out.ap(), in_=y_sb[:])
            else:
                nc.sync.dma_start(out=out.ap(), in_=reps_sb[:, 0:D])

    nc.compile()
    return nc


def _prep_in_maps(inputs):
    f32 = np.float32

    W3 = np.asarray(inputs["W3"], dtype=f32)
    WvT = np.ascontiguousarray(np.asarray(inputs["Wv"], dtype=f32).T)
    bv = np.asarray(inputs["bv"], dtype=f32)
    WtT = np.asarray(inputs["Wt"], dtype=f32).T          # [768, 512]
    WkT = np.asarray(inputs["Wk"], dtype=f32).T          # [1024, 512]
    WoT = np.asarray(inputs["Wo"], dtype=f32).T          # [512, 512]
    textT = np.asarray(inputs["text_features"], dtype=f32).T
    knowT = np.asarray(inputs["knowledge_features"], dtype=f32).T
    visT = np.asarray(inputs["visual_features"], dtype=f32).T
    bt = np.asarray(inputs["bt"], dtype=f32)
    bk = np.asarray(inputs["bk"], dtype=f32)
    bo = np.asarray(inputs["bo"], dtype=f32)
    gamma = np.asarray(inputs["gamma"], dtype=f32)
    beta = np.asarray(inputs["beta"], dtype=f32)

    def part(x, nc_, w):  # [nc_*128, w] -> [128, nc_*w] (ct-major cols)
        return np.ascontiguousarray(
            x.reshape(nc_, 128, w).transpose(1, 0, 2).reshape(128, nc_ * w))

    aux_shared = [
        part(WtT, 6, 512),
        part(WkT, 8, 512),
        None,  # per-core WvT slice
        part(WoT, 4, 512),
        WoT.sum(axis=1).reshape(4, 128).T,          # col-sums by (p, it)
        part(textT, 6, B),
        part(knowT, 8, B),
        part(visT, 16, B),
    ]

    smalls_shared = np.zeros((128, 28), f32)
    smalls_shared[:, 0:4] = bt.reshape(4, 128).T
    smalls_shared[:, 4:8] = bk.reshape(4, 128).T
    for lt in range(4):
        for b in range(B):
            smalls_shared[lt * B + b, 8 + b] = 1.0    # E matrix
    smalls_shared[0:B, 25] = bo.sum()

    reps = np.concatenate([
        np.tile(bo.reshape(1, D), (B, 1)),
        np.tile(gamma.reshape(1, D), (B, 1)),
        np.tile(beta.reshape(1, D), (B, 1)),
    ], axis=1).astype(f32)

    in_maps = []
    for m in range(NCORES):
        sl = slice(DSH * m, DSH * (m + 1))
        per = {"reps": reps}
        # W3 blocks: [i, p, (jt, lt), l2]
        Sblk = (W3[sl].reshape(DSH, 4, 128, 4, 128)
                .transpose(0, 2, 1, 3, 4).reshape(DSH, 128, 16, 128))
        Sblk = Sblk * W3SCALE
        if NB:
            idx = [jt * 4 + lt for (jt, lt) in BF_BLOCKS]
            per["w3bf"] = np.ascontiguousarray(
                Sblk[:, :, idx, :]).astype(BF16).reshape(NG, G, 128, NB * 128)
        if NF:
            idx = [jt * 4 + lt for (jt, lt) in F8_BLOCKS]
            per["w3f8"] = np.ascontiguousarray(
                Sblk[:, :, idx, :]).astype(F8NP).reshape(NG, G, 128, NF * 128)
        auxl = list(aux_shared)
        auxl[2] = part(WvT[:, sl] * np.float32(1.0 / W3SCALE), 16, DSH)
        per["aux"] = np.concatenate(auxl, axis=1).astype(BF16)
        sm = smalls_shared.copy()
        sm[0:DSH, 24] = bv[sl] * np.float32(1.0 / W3SCALE)
        per["smalls"] = sm
        in_maps.append(per)
    return in_maps


def kernel(**inputs):
    from concourse.bass_utils import run_bass_kernel_spmd

    key = (NF, F8, G)
    if key not in _CACHE:
        _CACHE[key] = _build_module()
    nc = _CACHE[key]

    in_maps = _prep_in_maps(inputs)
    trace = os.environ.get("KERNEL_TRACE", "0") == "1"
    res = run_bass_kernel_spmd(nc, in_maps, core_ids=list(range(NCORES)),
                               trace=trace)
    LAST["exec_time_ns"] = res.exec_time_ns
    LAST["results"] = res
    return np.asarray(res.results[0]["out"], dtype=np.float32)


# revision 76
# speedup vs baseline: 1.8888x; 1.0683x over previous
"""Trainium2 Bass kernel for KnowledgeAugmentedFusion.

  v = visual @ Wv.T + bv                      [B, D]
  t = text @ Wt.T + bt                        [B, D]
  k = knowledge @ Wk.T + bk                   [B, D]
  s = einsum('bj,ijl,bl->bi', t, W3, k)       [B, D]   (W3: [D, D, D])
  out = LayerNorm((v * s) @ Wo.T + bo)        [B, D]

Sharding: W3 along output-channel axis i across 8 cores (64 rows each);
fused = v*s slices are AllGathered, every core runs the small
output-layer + LayerNorm epilogue redundantly (per the sharding hint).

Dataflow (per core, per output channel i):
  stage 1:  psT[l, b] = sum_j W3[i, j, l] * t[b, j]
            -- W3 [128j x 128l] blocks are the STATIONARY matmul operand,
               tT [128j, 16b] the moving one; psT accumulates in one PSUM
               bank as 4 l-tile groups (onto a memset bank, start=False).
  stage 2:  prod[l, b] = psT[l, b] * kT[l, b]        (one DVE op, [128, 64])
  stage 3:  S4[(lt,b), i] = sum_p prod[p, (lt,b)]    (ones-matmul partition
            reduce into a persistent PSUM bank)
  end:      sT[i, b] = sum_lt S4[(lt,b), i]          (constant E matmul)
            fusedT = sT * vT, AllGather, epilogue matmul + LayerNorm.

W3 is streamed as a mix of bf16 and fp8 blocks (NF of the 16 [128x128]
(jt,lt) blocks per i are fp8): fp8 halves HBM traffic for those blocks;
the fp8 rounding error is tuned against the rel-err budget. A global
power-of-2 scale keeps fp8 values in the normal range; it is folded
into v on the host.
"""

import os
import sys

if "/opt/trn_rl_repo" not in sys.path:
    sys.path.insert(0, "/opt/trn_rl_repo")

import numpy as np
import ml_dtypes

B = 16
VD, TD, KD, D = 2048, 768, 1024, 512
NCORES = 8
DSH = D // NCORES  # 64 output channels per core
LN_EPS = 1e-5

BF16 = ml_dtypes.bfloat16

# --- W3 streaming config (env-overridable for experiments) --------------
NF = int(os.environ.get("K_NF", "16"))     # fp8 blocks out of 16 per i
NB = 16 - NF
F8 = os.environ.get("K_F8", "e3")          # e3 -> e3m4, e4 -> e4m3
G = int(os.environ.get("K_G", "2"))        # i-channels per W3 DMA
NG = DSH // G
W3SCALE = 256.0 if F8 == "e3" else 64.0
F8NP = ml_dtypes.float8_e3m4 if F8 == "e3" else ml_dtypes.float8_e4m3

# block order: first NB blocks bf16, last NF fp8 ((jt, lt) lexicographic)
ALL_BLOCKS = [(jt, lt) for jt in range(4) for lt in range(4)]
BF_BLOCKS = ALL_BLOCKS[:NB]
F8_BLOCKS = ALL_BLOCKS[NB:]
BF_POS = {b: n for n, b in enumerate(BF_BLOCKS)}
F8_POS = {b: n for n, b in enumerate(F8_BLOCKS)}

# aux1: needed before/during the W3 stream (bf16, [128, AUX1W]).
# Wk's upper 4 contraction chunks ride separately as e3m4 (x64 scale,
# with the matching knowledge.T rows pre-divided by 64).
KC8 = 4                         # Wk c-chunks (of 8) stored as fp8
TC8 = 3                         # Wt c-chunks (of 6) stored as fp8
OFF_WT = 0                      # [128, 6-TC8, 512]  Wt.T low chunks
OFF_WK = OFF_WT + (6 - TC8) * 512   # [128, 8-KC8, 512]  Wk.T low chunks
OFF_WV = OFF_WK + (8 - KC8) * 512   # [128, 16, 64]  Wv.T slice / W3SCALE
OFF_TX = OFF_WV + 16 * DSH      # [128, 6, 16]   text.T
OFF_KN = OFF_TX + 6 * B         # [128, 8, 16]   knowledge.T
OFF_VS = OFF_KN + 8 * B         # [128, 16, 16]  visual.T
AUX1W = OFF_VS + 16 * B
WK8SCALE = 64.0
# aux2: epilogue-only, loaded after the W3 stream (hidden under the tail)
OFF_WO = 0                      # [128, 4, 512]  Wo.T by (it, c)
OFF_WOS = OFF_WO + 4 * 512      # [128, 4]       col-sums of Wo.T by (it)
AUX2W = OFF_WOS + 4

_CACHE = {}
LAST = {}


def _build_module():
    n_i = int(os.environ.get("K_NI", str(DSH)))
    use_cc = os.environ.get("K_CC", "1") == "1"
    use_epi = os.environ.get("K_EPI", "1") == "1"
    w3_bufs = int(os.environ.get("K_W3BUFS", "4"))
    chunk_at = int(os.environ.get("K_CHUNK", "99"))
    late_aux = os.environ.get("K_LATEAUX", "0") == "1"
    n_junk = int(os.environ.get("K_JUNK", "0"))
    from concourse import bacc, tile, mybir

    fp32 = mybir.dt.float32
    bf16 = mybir.dt.bfloat16
    f8 = mybir.dt.float8e3 if F8 == "e3" else mybir.dt.float8e4
    OP = mybir.AluOpType
    ACT = mybir.ActivationFunctionType

    nc = bacc.Bacc("TRN2", target_bir_lowering=False, debug=False,
                   num_devices=NCORES)

    # ---- DRAM I/O ----------------------------------------------------
    XB = NB * 128
    XF = NF * 128
    if NB:
        w3bf = nc.dram_tensor("w3bf", [NG, G, 128, XB], bf16,
                              kind="ExternalInput")
    if NF:
        w3f8 = nc.dram_tensor("w3f8", [NG, G, 128, XF], f8,
                              kind="ExternalInput")
    aux = nc.dram_tensor("aux", [128, AUX1W], bf16, kind="ExternalInput")
    wk8 = nc.dram_tensor("wk8", [128, KC8 * 512], f8, kind="ExternalInput")
    wt8 = nc.dram_tensor("wt8", [128, TC8 * 512], f8, kind="ExternalInput")
    aux2 = nc.dram_tensor("aux2", [128, AUX2W], bf16, kind="ExternalInput")
    smalls = nc.dram_tensor("smalls", [128, 28], fp32, kind="ExternalInput")
    reps = nc.dram_tensor("reps", [B, 3 * D], bf16, kind="ExternalInput")
    out = nc.dram_tensor("out", [B, D], fp32, kind="ExternalOutput")

    with tile.TileContext(nc) as tc:
        with tc.tile_pool(name="const", bufs=1) as constp, \
             tc.tile_pool(name="w3p", bufs=w3_bufs) as w3p, \
             tc.tile_pool(name="f8p", bufs=w3_bufs) as f8p, \
             tc.tile_pool(name="scr", bufs=2) as scrp, \
             tc.tile_pool(name="pp", bufs=2, space="PSUM") as pp, \
             tc.tile_pool(name="psq", bufs=2, space="PSUM") as psq, \
             tc.tile_pool(name="ps4", bufs=1, space="PSUM") as ps4, \
             tc.tile_pool(name="ppx", bufs=1, space="PSUM") as ppx, \
             tc.tile_pool(name="ppm", bufs=1, space="PSUM") as ppm, \
             tc.tile_pool(name="pst", bufs=1, space="PSUM") as pst, \
             tc.tile_pool(name="dram", bufs=1, space="DRAM") as dramp:

            # ---- aux / constants into SBUF ---------------------------
            aux_sb = constp.tile([128, AUX1W], bf16)
            nc.sync.dma_start(out=aux_sb[:], in_=aux.ap())
            wk8_sb = constp.tile([128, KC8 * 512], f8)
            nc.sync.dma_start(out=wk8_sb[:], in_=wk8.ap())
            wt8_sb = constp.tile([128, TC8 * 512], f8)
            nc.sync.dma_start(out=wt8_sb[:], in_=wt8.ap())
            smalls_sb = constp.tile([128, 28], fp32)
            nc.sync.dma_start(out=smalls_sb[:], in_=smalls.ap())
            aux2_sb = constp.tile([128, AUX2W], bf16)
            reps_sb = constp.tile([B, 3 * D], bf16)
            if not late_aux:
                nc.sync.dma_start(out=aux2_sb[:], in_=aux2.ap())
                nc.sync.dma_start(out=reps_sb[:], in_=reps.ap())

            ones_sb = constp.tile([128, 1], fp32)
            nc.vector.memset(ones_sb[:], 1.0)
            ones16 = constp.tile([1, B], bf16)
            nc.vector.memset(ones16[:], 1.0)

            # warm the activation table (sqrt_and_others holds Sqrt +
            # Square + Copy) so the load hides under the aux DMA
            warm_t = constp.tile([1, 1], fp32)
            nc.scalar.activation(out=warm_t[:], in_=ones_sb[0:1, 0:1],
                                 func=ACT.Sqrt)

            # ---- prologue: tT (bf16), kT (fp32), vT (fp32) -----------
            tT_sb = constp.tile([128, 4 * B], bf16)
            for jt in range(4):
                pt = pp.tile([128, B], fp32, tag="pp")
                for ct in range(6):
                    if ct < 6 - TC8:
                        lhsT = aux_sb[:, OFF_WT + ct * 512 + jt * 128:
                                      OFF_WT + ct * 512 + jt * 128 + 128]
                    else:
                        c8 = ct - (6 - TC8)
                        lhsT = wt8_sb[:, c8 * 512 + jt * 128:
                                      c8 * 512 + jt * 128 + 128]
                    nc.tensor.matmul(
                        out=pt[:],
                        lhsT=lhsT,
                        rhs=aux_sb[:, OFF_TX + ct * B: OFF_TX + ct * B + B],
                        start=(ct == 0), stop=(ct == 5))
                nc.vector.tensor_scalar(
                    out=tT_sb[:, B * jt: B * jt + B], in0=pt[:],
                    scalar1=smalls_sb[:, jt: jt + 1], scalar2=None,
                    op0=OP.add)

            kT_sb = constp.tile([128, 4 * B], fp32)
            for lt in range(4):
                pk = pp.tile([128, B], fp32, tag="pp")
                for ct in range(8):
                    if ct < 8 - KC8:
                        lhsT = aux_sb[:, OFF_WK + ct * 512 + lt * 128:
                                      OFF_WK + ct * 512 + lt * 128 + 128]
                    else:
                        c8 = ct - (8 - KC8)
                        lhsT = wk8_sb[:, c8 * 512 + lt * 128:
                                      c8 * 512 + lt * 128 + 128]
                    nc.tensor.matmul(
                        out=pk[:],
                        lhsT=lhsT,
                        rhs=aux_sb[:, OFF_KN + ct * B: OFF_KN + ct * B + B],
                        start=(ct == 0), stop=(ct == 7))
                nc.vector.tensor_scalar(
                    out=kT_sb[:, B * lt: B * lt + B], in0=pk[:],
                    scalar1=smalls_sb[:, 4 + lt: 5 + lt], scalar2=None,
                    op0=OP.add)

            pv = pp.tile([DSH, B], fp32, tag="pp")
            for ct in range(16):
                nc.tensor.matmul(
                    out=pv[:],
                    lhsT=aux_sb[:, OFF_WV + ct * DSH: OFF_WV + ct * DSH + DSH],
                    rhs=aux_sb[:, OFF_VS + ct * B: OFF_VS + ct * B + B],
                    start=(ct == 0), stop=(ct == 15))
            vsb = constp.tile([DSH, B], fp32)
            nc.vector.tensor_scalar(
                out=vsb[:], in0=pv[:], scalar1=smalls_sb[0:DSH, 24:25],
                scalar2=None, op0=OP.add)

            # ---- persistent S4 accumulator [64 (lt,b), 64 i] ---------
            S4 = ps4.tile([64, 64], fp32)
            nc.vector.memset(S4[:], 0.0)

            # fused chunk pipeline: half of s -> fused -> cc_in overlaps
            # the W3 stream; only the second half sits in the tail.
            S4sb = constp.tile([64, 64], fp32)
            sTt = pst.tile([64, B], fp32)
            fusedT_bf = constp.tile([DSH, B], bf16)
            cc_in = dramp.tile([DSH, B], bf16)
            chunks_done = [0]

            # cc_in writes and the gather go on the gpsimd queue: DRAM is
            # not dependency-managed by the tile framework, so same-queue
            # program order is what serializes write -> collective -> read.
            nc.vector.memset(sTt[:], 0.0)

            def fused_chunk(c):
                r0, r1 = 32 * c, 32 * (c + 1)
                nc.vector.tensor_copy(S4sb[:, r0:r1], S4[:, r0:r1])
                nc.tensor.matmul(out=sTt[r0:r1, :], lhsT=S4sb[:, r0:r1],
                                 rhs=smalls_sb[0:64, 8:24],
                                 start=False, stop=True,
                                 skip_group_check=True)
                nc.vector.tensor_tensor(out=fusedT_bf[r0:r1, :],
                                        in0=sTt[r0:r1, :], in1=vsb[r0:r1, :],
                                        op=OP.mult)
                nc.gpsimd.dma_start(out=cc_in[:][r0:r1],
                                    in_=fusedT_bf[r0:r1, :])
                chunks_done[0] = c + 1

            # ---- main loop over output channels ----------------------
            for g in range(NG):
                if g * G >= n_i:
                    break
                split_last = (g == NG - 1 and G > 1 and
                              os.environ.get("K_SPLITLAST", "0") == "1")
                if NB:
                    if split_last:
                        wbf_s = []
                        for gi in range(G):
                            tb = w3p.tile([128, XB], bf16, tag=f"wbf{gi}")
                            nc.sync.dma_start(out=tb[:],
                                              in_=w3bf.ap()[g][gi])
                            wbf_s.append(tb)
                    else:
                        wbf = w3p.tile([128, G * XB], bf16, tag="wbf")
                        nc.sync.dma_start(
                            out=wbf[:].rearrange("p (g x) -> p g x", g=G),
                            in_=w3bf.ap()[g].rearrange("g p x -> p g x"))
                if NF:
                    if split_last:
                        wf8_s = []
                        for gi in range(G):
                            tf = f8p.tile([128, XF], f8, tag=f"wf8{gi}")
                            nc.sync.dma_start(out=tf[:],
                                              in_=w3f8.ap()[g][gi])
                            wf8_s.append(tf)
                    else:
                        wf8 = f8p.tile([128, G * XF], f8, tag="wf8")
                        nc.sync.dma_start(
                            out=wf8[:].rearrange("p (g x) -> p g x", g=G),
                            in_=w3f8.ap()[g].rearrange("g p x -> p g x"))
                for gi in range(G):
                    i = g * G + gi
                    psT = psq.tile([128, 64], fp32, tag="psT")
                    nc.vector.memset(psT[:], 0.0)
                    for lt in range(4):
                        for jt in range(4):
                            if (jt, lt) in BF_POS:
                                if split_last:
                                    col = BF_POS[(jt, lt)] * 128
                                    lhsT = wbf_s[gi][:, col: col + 128]
                                else:
                                    col = (gi * NB + BF_POS[(jt, lt)]) * 128
                                    lhsT = wbf[:, col: col + 128]
                            else:
                                if split_last:
                                    col = F8_POS[(jt, lt)] * 128
                                    lhsT = wf8_s[gi][:, col: col + 128]
                                else:
                                    col = (gi * NF + F8_POS[(jt, lt)]) * 128
                                    lhsT = wf8[:, col: col + 128]
                            nc.tensor.matmul(
                                out=psT[:, lt * B: lt * B + B],
                                lhsT=lhsT,
                                rhs=tT_sb[:, B * jt: B * jt + B],
                                start=False, stop=True,
                                skip_group_check=True)
                    prod = scrp.tile([128, 64], fp32, tag="prod")
                    nc.vector.tensor_tensor(out=prod[:], in0=psT[:],
                                            in1=kT_sb[:], op=OP.mult)
                    nc.tensor.matmul(
                        out=S4[:, i: i + 1], lhsT=prod[:], rhs=ones_sb[:],
                        start=False, stop=True, skip_group_check=True)
                    last_prod = prod
                if (g + 1) * G == 48:
                    # pre-copy finished S4 columns off the tail critical path
                    nc.vector.tensor_copy(S4sb[:, 0:48], S4[:, 0:48])
                if (g + 1) * G == chunk_at:
                    fused_chunk(0)

            # ---- epilogue-only weights, loaded after the W3 stream ---
            if late_aux:
                nc.sync.dma_start(out=aux2_sb[:], in_=aux2.ap())
                nc.sync.dma_start(out=reps_sb[:], in_=reps.ap())

            # ---- s -> fused -> AllGather -----------------------------
            if chunks_done[0] == 0:
                # single-shot: one E-matmul + one fused multiply + one cc DMA
                nc.vector.tensor_copy(S4sb[:, 48:64], S4[:, 48:64])
                nc.tensor.matmul(out=sTt[:], lhsT=S4sb[:],
                                 rhs=smalls_sb[0:64, 8:24],
                                 start=False, stop=True,
                                 skip_group_check=True)
                nc.vector.tensor_tensor(out=fusedT_bf[:], in0=sTt[:],
                                        in1=vsb[:], op=OP.mult)
                nc.gpsimd.dma_start(out=cc_in[:], in_=fusedT_bf[:])
            else:
                fused_chunk(1)

            # keep the PE p-state ramped through the cc/gather DMA window
            # so the epilogue matmuls run at full clock; reading last_prod
            # pins these after the main loop (the scheduler cannot hoist)
            if n_junk:
                junkp = ppx.tile([B, D], fp32, tag="ppx")
                for _ in range(n_junk):
                    nc.tensor.matmul(out=junkp[:, 0:64],
                                     lhsT=last_prod[:, 0:B],
                                     rhs=last_prod[:], start=True, stop=True)

            cc_out = dramp.tile([NCORES, DSH, B], bf16)
            if use_cc:
                nc.gpsimd.collective_compute(
                    "AllGather", OP.bypass,
                    replica_groups=[list(range(NCORES))],
                    ins=[cc_in.opt()], outs=[cc_out.opt()])

            # fusedT [512 i, 16 b] as [128, (4 it, 16 b)]; i = 128*it + p
            fusedT_sb = constp.tile([128, 4 * B], bf16)
            nc.gpsimd.dma_start(
                out=fusedT_sb[:].rearrange("p (it b) -> p it b", it=4),
                in_=cc_out[:].rearrange("(it c2) r b -> (c2 r) it b", it=4))

            if use_epi:
                # ---- epilogue: out = LN(fused @ Wo.T + bo) -----------
                ps_x = ppx.tile([B, D], fp32, tag="ppx")
                ps_mu = ppm.tile([B, 1], fp32, tag="ppm")
                # K=1 matmul adds bo (broadcast row) into the x accumulation
                nc.tensor.matmul(out=ps_x[:], lhsT=ones16[:],
                                 rhs=reps_sb[0:1, 0:D], start=True, stop=False)
                for it in range(4):
                    nc.tensor.matmul(
                        out=ps_x[:],
                        lhsT=fusedT_sb[:, B * it: B * it + B],
                        rhs=aux2_sb[:, OFF_WO + it * 512: OFF_WO + it * 512 + 512],
                        start=False, stop=(it == 3))
                for it in range(4):
                    nc.tensor.matmul(
                        out=ps_mu[:],
                        lhsT=fusedT_sb[:, B * it: B * it + B],
                        rhs=aux2_sb[:, OFF_WOS + it: OFF_WOS + it + 1],
                        start=(it == 0), stop=(it == 3))

                eps_t = constp.tile([B, 1], fp32)
                nc.vector.memset(eps_t[:], LN_EPS)

                # x (= fused @ Wo.T + bo) lives in ps_x; mean via colsum
                mean_t = constp.tile([B, 1], fp32)
                nc.vector.tensor_scalar(
                    out=mean_t[:], in0=ps_mu[:],
                    scalar1=smalls_sb[0:B, 25:26], scalar2=1.0 / D,
                    op0=OP.add, op1=OP.mult)
                # var = E[x^2] - mean^2
                sq_junk = scrp.tile([B, D], fp32, tag="sqj")
                sqs_t = constp.tile([B, 1], fp32)
                nc.scalar.activation(out=sq_junk[:], in_=ps_x[:],
                                     func=ACT.Square, accum_out=sqs_t[:])
                mu2_t = constp.tile([B, 1], fp32)
                nc.scalar.activation(out=mu2_t[:], in_=mean_t[:],
                                     func=ACT.Square)
                var_t = constp.tile([B, 1], fp32)
                nc.vector.scalar_tensor_tensor(
                    out=var_t[:], in0=sqs_t[:], scalar=1.0 / D,
                    in1=mu2_t[:], op0=OP.mult, op1=OP.subtract)
                std_t = constp.tile([B, 1], fp32)
                nc.scalar.activation(out=std_t[:], in_=var_t[:],
                                     func=ACT.Sqrt, bias=eps_t[:])
                rstd_t = constp.tile([B, 1], fp32)
                nc.vector.reciprocal(out=rstd_t[:], in_=std_t[:])
                # y = (x - mean) * gamma * rstd + beta
                xn_sb = scrp.tile([B, D], fp32, tag="xn")
                nc.vector.scalar_tensor_tensor(
                    out=xn_sb[:], in0=ps_x[:], scalar=mean_t[:],
                    in1=reps_sb[:, D:2 * D], op0=OP.subtract, op1=OP.mult)
                y_sb = scrp.tile([B, D], fp32, tag="y")
                nc.vector.scalar_tensor_tensor(
                    out=y_sb[:], in0=xn_sb[:], scalar=rstd_t[:],
                    in1=reps_sb[:, 2 * D:3 * D], op0=OP.mult, op1=OP.add)
                nc.sync.dma_start(out=out.ap(), in_=y_sb[:])
            else:
                nc.sync.dma_start(out=out.ap(), in_=reps_sb[:, 0:D])

    nc.compile()
    return nc


def _prep_in_maps(inputs):
    f32 = np.float32

    W3 = np.asarray(inputs["W3"], dtype=f32)
    WvT = np.ascontiguousarray(np.asarray(inputs["Wv"], dtype=f32).T)
    bv = np.asarray(inputs["bv"], dtype=f32)
    WtT = np.asarray(inputs["Wt"], dtype=f32).T          # [768, 512]
    WkT = np.asarray(inputs["Wk"], dtype=f32).T          # [1024, 512]
    WoT = np.asarray(inputs["Wo"], dtype=f32).T          # [512, 512]
    textT = np.asarray(inputs["text_features"], dtype=f32).T
    knowT = np.asarray(inputs["knowledge_features"], dtype=f32).T
    visT = np.asarray(inputs["visual_features"], dtype=f32).T
    bt = np.asarray(inputs["bt"], dtype=f32)
    bk = np.asarray(inputs["bk"], dtype=f32)
    bo = np.asarray(inputs["bo"], dtype=f32)
    gamma = np.asarray(inputs["gamma"], dtype=f32)
    beta = np.asarray(inputs["beta"], dtype=f32)

    def part(x, nc_, w):  # [nc_*128, w] -> [128, nc_*w] (ct-major cols)
        return np.ascontiguousarray(
            x.reshape(nc_, 128, w).transpose(1, 0, 2).reshape(128, nc_ * w))

    ksplit = (8 - KC8) * 128
    knowT_adj = knowT.copy()
    knowT_adj[ksplit:] *= np.float32(1.0 / WK8SCALE)
    wk8_all = part(WkT[ksplit:] * np.float32(WK8SCALE), KC8, 512).astype(F8NP)
    tsplit = (6 - TC8) * 128
    textT_adj = textT.copy()
    textT_adj[tsplit:] *= np.float32(1.0 / WK8SCALE)
    wt8_all = part(WtT[tsplit:] * np.float32(WK8SCALE), TC8, 512).astype(F8NP)
    aux_shared = [
        part(WtT[:tsplit], 6 - TC8, 512),
        part(WkT[:ksplit], 8 - KC8, 512),
        None,  # per-core WvT slice
        part(textT_adj, 6, B),
        part(knowT_adj, 8, B),
        part(visT, 16, B),
    ]
    aux2_all = np.concatenate([
        part(WoT, 4, 512),
        WoT.sum(axis=1).reshape(4, 128).T,          # col-sums by (p, it)
    ], axis=1).astype(BF16)

    smalls_shared = np.zeros((128, 28), f32)
    smalls_shared[:, 0:4] = bt.reshape(4, 128).T
    smalls_shared[:, 4:8] = bk.reshape(4, 128).T
    for lt in range(4):
        for b in range(B):
            smalls_shared[lt * B + b, 8 + b] = 1.0    # E matrix
    smalls_shared[0:B, 25] = bo.sum()

    reps = np.concatenate([
        np.tile(bo.reshape(1, D), (B, 1)),
        np.tile(gamma.reshape(1, D), (B, 1)),
        np.tile(beta.reshape(1, D), (B, 1)),
    ], axis=1).astype(BF16)

    in_maps = []
    for m in range(NCORES):
        sl = slice(DSH * m, DSH * (m + 1))
        per = {"reps": reps}
        # W3 blocks: [i, p, (jt, lt), l2]
        Sblk = (W3[sl].reshape(DSH, 4, 128, 4, 128)
                .transpose(0, 2, 1, 3, 4).reshape(DSH, 128, 16, 128))
        Sblk = Sblk * W3SCALE
        if NB:
            idx = [jt * 4 + lt for (jt, lt) in BF_BLOCKS]
            per["w3bf"] = np.ascontiguousarray(
                Sblk[:, :, idx, :]).astype(BF16).reshape(NG, G, 128, NB * 128)
        if NF:
            idx = [jt * 4 + lt for (jt, lt) in F8_BLOCKS]
            per["w3f8"] = np.ascontiguousarray(
                Sblk[:, :, idx, :]).astype(F8NP).reshape(NG, G, 128, NF * 128)
        auxl = list(aux_shared)
        auxl[2] = part(WvT[:, sl] * np.float32(1.0 / W3SCALE), 16, DSH)
        per["aux"] = np.concatenate(auxl, axis=1).astype(BF16)
        per["aux2"] = aux2_all
        per["wk8"] = wk8_all
        per["wt8"] = wt8_all
        sm = smalls_shared.copy()
        sm[0:DSH, 24] = bv[sl] * np.float32(1.0 / W3SCALE)
        per["smalls"] = sm
        in_maps.append(per)
    return in_maps


def kernel(**inputs):
    from concourse.bass_utils import run_bass_kernel_spmd

    key = (NF, F8, G)
    if key not in _CACHE:
        _CACHE[key] = _build_module()
    nc = _CACHE[key]

    in_maps = _prep_in_maps(inputs)
    trace = os.environ.get("KERNEL_TRACE", "0") == "1"
    res = run_bass_kernel_spmd(nc, in_maps, core_ids=list(range(NCORES)),
                               trace=trace)
    LAST["exec_time_ns"] = res.exec_time_ns
    LAST["results"] = res
    return np.asarray(res.results[0]["out"], dtype=np.float32)
